# revision 29
# baseline (speedup 1.0000x reference)
"""nn_MemoryAttention TP8 Trainium2 kernel (v1.1).

8 NeuronCores, Megatron tensor-parallel, T-layout activations [feature, token]
(512 token cols = 4 batch x 128). Per-core shards: wq/wk/wv/wkm/wvm col-split
by heads (2/core), wm/wo col-split (256 feat), w1/w3 col-split (704->768 pad),
w2 row-split. Per chunk: wm -> AG(f16) -> FFN (rmsnorm folded into per-token
scale that commutes through the matmuls) -> AR(f16) -> mem K/V + RoPE ->
attention (chunk queries only; norm2 scale applied to score/att columns) ->
AG(f16) = next om. RoPE via head-dim permutation baked into weight columns.
Matmuls float32r; collective payloads float16; f16->f32r casts on the idle
Pool/Activation engines.
"""
import numpy as np

DIM = 2048; NH = 16; HD = 128; MEM = 128; SEQ = 2048; B = 4; HID = 5632
EPS = 1e-5
NC = 8
HPC = NH // NC          # 2 heads per core
FPC = DIM // NC         # 256
HIDP = 768              # padded per-core hidden
NCH = SEQ // MEM        # 16 chunks
TOK = B * MEM           # 512 token columns
KT = DIM // 128         # 16 feature k-tiles
HKT = HIDP // 128       # 6 hidden k-tiles

_RUNTIME = {}
import os as _os
COLL = _os.environ.get("KERNEL_NO_COLL", "") != "1"   # diagnostic switch


def _trunc22(x):
    # round-to-nearest to fp22 (13 mantissa bits); HW fp32r truncation then
    # leaves these values unchanged, so host and device agree.
    u = np.ascontiguousarray(x, np.float32).view(np.uint32)
    return ((u + np.uint32(0x200)) & np.uint32(0xFFFFFC00)).view(np.float32)


def _head_perm():
    p = np.concatenate([np.arange(0, HD, 2), np.arange(1, HD, 2)])
    return np.concatenate([h * HD + p for h in range(NH)])


def _slab(w):
    """[2048, C] -> [C//128, 128, KT, 128]."""
    C = w.shape[1]
    return np.ascontiguousarray(
        w.reshape(KT, 128, C // 128, 128).transpose(2, 1, 0, 3))


def _slab_w2(w2):
    """[768, 2048] -> [16, 128, HKT, 128]."""
    return np.ascontiguousarray(
        w2.reshape(HKT, 128, 16, 128).transpose(2, 1, 0, 3))


def _prepare(inputs):
    perm = _head_perm()
    scale = 1.0 / np.sqrt(HD)
    fwv = np.asarray(inputs["ffn_norm_w"], np.float32)
    mwv = np.asarray(inputs["mem_norm_w"], np.float32)
    wq = np.asarray(inputs["wq"])[:, perm] * scale
    wk = np.asarray(inputs["wk"])[:, perm]
    # norm2 weight folded into wkm/wvm rows (scale per token applied on-chip)
    wkm = (mwv[:, None] * np.asarray(inputs["wkm"]))[:, perm]
    wv = np.asarray(inputs["wv"])
    wvm = mwv[:, None] * np.asarray(inputs["wvm"])
    wm = np.asarray(inputs["wm"]); wo = np.asarray(inputs["wo"])
    w1 = np.zeros((DIM, NC * HIDP), np.float32)
    w3 = np.zeros((DIM, NC * HIDP), np.float32)
    w2 = np.zeros((NC * HIDP, DIM), np.float32)
    for c in range(NC):
        w1[:, c * HIDP:c * HIDP + 704] = np.asarray(inputs["w1"])[:, c * 704:(c + 1) * 704]
        w3[:, c * HIDP:c * HIDP + 704] = np.asarray(inputs["w3"])[:, c * 704:(c + 1) * 704]
        w2[c * HIDP:c * HIDP + 704] = np.asarray(inputs["w2"])[c * 704:(c + 1) * 704]
    # norm1 weight folded into w1/w3 rows
    w1 = fwv[:, None] * w1
    w3 = fwv[:, None] * w3
    fc = np.asarray(inputs["freqs_cos"]); fs = np.asarray(inputs["freqs_sin"])
    cs_mem = np.concatenate([np.tile(fc[0:MEM].T, (1, B)),
                             np.tile(fs[0:MEM].T, (1, B))], 0).astype(np.float16)
    cs_x = np.concatenate([np.tile(fc[MEM:2 * MEM].T, (1, B)),
                           np.tile(fs[MEM:2 * MEM].T, (1, B))], 0).astype(np.float16)
    mask = np.zeros((MEM, MEM), np.float16)
    for i in range(MEM):
        mask[i, i + 1:] = -60000.0
    mask = np.tile(mask, (1, B))
    x = np.asarray(inputs["x"])
    xT = x.reshape(B, NCH, MEM, DIM).transpose(1, 3, 0, 2).reshape(
        NCH, DIM, TOK).astype(np.float16)
    om0 = np.asarray(inputs["origin_mem"])
    omT0 = _trunc22(om0.transpose(2, 0, 1).reshape(DIM, TOK))
    in_maps = []
    for c in range(NC):
        hsl = slice(c * FPC, (c + 1) * FPC)
        hidsl = slice(c * HIDP, (c + 1) * HIDP)
        in_maps.append({
            "WQ": _trunc22(_slab(wq[:, hsl])),
            "WK": _trunc22(_slab(wk[:, hsl])),
            "WKM": _trunc22(_slab(wkm[:, hsl])),
            "WM": _trunc22(_slab(wm[:, hsl])),
            "WO": _trunc22(_slab(wo[:, hsl])),
            "W1": _trunc22(_slab(w1[:, hidsl])),
            "W3": _trunc22(_slab(w3[:, hidsl])),
            "W2": _trunc22(_slab_w2(w2[hidsl, :])),
            "WV": _trunc22(np.ascontiguousarray(wv[:, hsl])),
            "WVM": _trunc22(np.ascontiguousarray(wvm[:, hsl])),
            "XT": xT, "OM0T": omT0,
            "CSM": cs_mem, "CSX": cs_x,
            "MASK": mask,
        })
    return in_maps


def _build():
    import concourse.bacc as bacc
    import concourse.tile as tile
    import concourse.mybir as mybir
    from concourse.masks import make_identity
    from contextlib import ExitStack

    dt = mybir.dt
    AluOp = mybir.AluOpType
    AFT = mybir.ActivationFunctionType
    f32, f32r, f16 = dt.float32, dt.float32r, dt.float16

    nc = bacc.Bacc("TRN2", target_bir_lowering=False, debug=False,
                   num_devices=NC)

    def din(name, shape, dtype=f32r):
        return nc.dram_tensor(name, shape, dtype, kind="ExternalInput")

    WQ = din("WQ", [2, 128, KT, 128]); WK = din("WK", [2, 128, KT, 128])
    WKM = din("WKM", [2, 128, KT, 128]); WM = din("WM", [2, 128, KT, 128])
    WO = din("WO", [2, 128, KT, 128])
    W1 = din("W1", [HKT, 128, KT, 128]); W3 = din("W3", [HKT, 128, KT, 128])
    W2 = din("W2", [KT, 128, HKT, 128])
    WV = din("WV", [DIM, FPC]); WVM = din("WVM", [DIM, FPC])
    XT = din("XT", [NCH, DIM, TOK], f16); OM0T = din("OM0T", [DIM, TOK])
    CSM = din("CSM", [128, TOK], f16); CSX = din("CSX", [128, TOK], f16)
    MASK = din("MASK", [MEM, TOK], f16)
    YO = nc.dram_tensor("YO", [NCH, FPC, TOK], f16, kind="ExternalOutput")

    rg = [list(range(NC))]
    SH = "Shared" if COLL else "Local"

    with tile.TileContext(nc) as tc:
        es = ExitStack()
        es.enter_context(nc.allow_low_precision(
            reason="f16 collective payloads validated within tolerance"))
        const = es.enter_context(tc.tile_pool(name="const", bufs=1))
        wslab = es.enter_context(tc.tile_pool(name="wslab", bufs=3))
        w2slab = es.enter_context(tc.tile_pool(name="w2slab", bufs=3))
        wrhs = es.enter_context(tc.tile_pool(name="wrhs", bufs=2))
        xpool = es.enter_context(tc.tile_pool(name="xpool", bufs=1))
        big = es.enter_context(tc.tile_pool(name="big", bufs=2))
        gpool = es.enter_context(tc.tile_pool(name="gpool", bufs=1))
        qkpool = es.enter_context(tc.tile_pool(name="qkpool", bufs=2))
        vpool = es.enter_context(tc.tile_pool(name="vpool", bufs=2))
        kmpool = es.enter_context(tc.tile_pool(name="kmpool", bufs=1))
        stg = es.enter_context(tc.tile_pool(name="stg", bufs=2))
        attp = es.enter_context(tc.tile_pool(name="attp", bufs=2))
        tmpp = es.enter_context(tc.tile_pool(name="tmpp", bufs=1))
        smol = es.enter_context(tc.tile_pool(name="smol", bufs=1))
        dram = es.enter_context(tc.tile_pool(name="dram", bufs=1, space="DRAM"))
        psA = es.enter_context(tc.tile_pool(name="psA", bufs=4, space="PSUM"))
        psS = es.enter_context(tc.tile_pool(name="psS", bufs=2, space="PSUM"))

        # ---- constants
        csm = const.tile([128, TOK], f16); nc.sync.dma_start(csm[:], CSM[:])
        csx = const.tile([128, TOK], f16); nc.sync.dma_start(csx[:], CSX[:])
        maskx = const.tile([MEM, TOK], f16)
        nc.sync.dma_start(maskx[:], MASK[:])
        scratch32 = const.tile([128, 128], f32)
        nc.vector.memset(scratch32[:], 1.0)
        ones = const.tile([128, 1], f32r)
        nc.vector.tensor_copy(ones[:], scratch32[:, 0:1])
        onesrow = const.tile([1, 128], f32r)
        nc.vector.tensor_copy(onesrow[:], scratch32[0:1, :])
        ident32 = const.tile([128, 128], f32)
        make_identity(nc, ident32)
        ident = const.tile([128, 128], f32r)
        nc.vector.tensor_copy(ident[:], ident32[:])
        epst = const.tile([1, 1], f32)
        nc.vector.memset(epst[:], EPS)

        om = big.tile([128, KT, TOK], f32r, tag="big", name="om_init")
        nc.sync.dma_start(om[:], OM0T[:].rearrange("(k p) t -> p k t", p=128))

        def mm(p, lhsT, rhs, start, stop):
            nc.tensor.matmul(p, lhsT, rhs, start=start, stop=stop)

        def proj_nslab(Wd, rhs_tile):
            outs = []
            for n in range(2):
                ws = wslab.tile([128, KT, 128], f32r, tag="wslab", name="ws")
                nc.sync.dma_start(ws[:], Wd[n])
                p = psA.tile([128, TOK], f32, tag="mm", name="pp")
                for k in range(KT):
                    mm(p[:], ws[:, k, :], rhs_tile[:, k, :], k == 0, k == KT - 1)
                outs.append(p)
            return outs

        def rope2(dst, src01, cs):
            cosT, sinT = cs[0:64, :], cs[64:128, :]
            for h in range(2):
                ph = src01[h]
                r, i = ph[0:64, :], ph[64:128, :]
                t1 = tmpp.tile([64, TOK], f32, tag="t1", name="t1")
                t2 = tmpp.tile([64, TOK], f32, tag="t2", name="t2")
                nc.vector.tensor_mul(t1[:], r, cosT)
                nc.vector.tensor_mul(t2[:], i, sinT)
                nc.vector.tensor_sub(dst[0:64, h, :], t1[:], t2[:])
                t3 = tmpp.tile([64, TOK], f32, tag="t1", name="t3")
                t4 = tmpp.tile([64, TOK], f32, tag="t2", name="t4")
                nc.vector.tensor_mul(t3[:], r, sinT)
                nc.vector.tensor_mul(t4[:], i, cosT)
                nc.vector.tensor_add(dst[64:128, h, :], t3[:], t4[:])

        def rowscale(src_tile, out_sb):
            """per-token 1/rms row [1, TOK] -> broadcast [128, TOK] in sbuf."""
            ssq = psS.tile([1, TOK], f32, tag="sc", name="ssq")
            for k in range(KT):
                sq = stg.tile([128, TOK], f32r, tag="scr", name="sq")
                nc.scalar.activation(sq[:], src_tile[:, k, :], AFT.Square)
                mm(ssq[:], ones[:], sq[:], k == 0, k == KT - 1)
            rstd = psS.tile([1, TOK], f32, tag="sc", name="rstd")
            nc.scalar.activation(rstd[:], ssq[:], AFT.Sqrt,
                                 bias=epst[:], scale=1.0 / DIM)
            rec_r = smol.tile([1, TOK], f32r, tag="recr", name="recr")
            nc.vector.reciprocal(rec_r[:], rstd[:])
            bc = psA.tile([128, TOK], f32, tag="mm", name="bc")
            mm(bc[:], onesrow[:], rec_r[:], True, True)
            nc.vector.tensor_copy(out_sb[:], bc[:])

        def vproj(lhs_tile, Wd, tag):
            """v (normal layout) [128 tok, B, FPC]; lhs = xk or om2."""
            v = vpool.tile([128, B, FPC], f32r, tag=tag, name=tag,
                           bufs=(2 if tag == "vx" else 1))
            pvs = [psS.tile([128, FPC], f32, tag=("sc" if b < 2 else "tr"),
                            name=f"pv{b}") for b in range(B)]
            for k in range(KT):
                wv_k = wrhs.tile([128, FPC], f32r, tag="wrhs", name="wvk")
                nc.scalar.dma_start(wv_k[:], Wd[k * 128:(k + 1) * 128, :])
                for b in range(B):
                    mm(pvs[b][:], lhs_tile[:, k, b * 128:(b + 1) * 128],
                       wv_k[:], k == 0, k == KT - 1)
            for b in range(B):
                nc.vector.tensor_copy(v[:, b, :], pvs[b][:])
            return v

        def x_side_qk(t):
            xk = xpool.tile([128, KT, TOK], f32r, tag="xk", name="xk")
            nc.gpsimd.dma_start(xk[:],
                                XT[t].rearrange("(k p) t2 -> p k t2", p=128))
            qps = proj_nslab(WQ, xk)
            qT = qkpool.tile([128, 2, TOK], f32r, tag="qT", name="qT")
            rope2(qT, qps, csx)
            return xk, qT

        def x_side_kv(xk):
            kps = proj_nslab(WK, xk)
            kxT = qkpool.tile([128, 2, TOK], f32r, tag="kxT", name="kxT")
            rope2(kxT, kps, csx)
            vx = vproj(xk, WV, "vx")
            return kxT, vx

        def allgather(in_d, out_d):
            if COLL:
                nc.gpsimd.collective_compute(
                    "AllGather", AluOp.bypass, replica_groups=rg,
                    ins=[in_d[:].opt()], outs=[out_d[:].opt()])
            else:
                n = out_d.shape[0] // in_d.shape[0]
                sz = in_d.shape[0]
                for r in range(n):
                    nc.sync.dma_start(out_d[r * sz:(r + 1) * sz, :], in_d[:])

        def allreduce(in_d, out_d):
            if COLL:
                nc.gpsimd.collective_compute(
                    "AllReduce", AluOp.add, replica_groups=rg,
                    ins=[in_d[:].opt()], outs=[out_d[:].opt()])
            else:
                nc.sync.dma_start(out_d[:], in_d[:])

        def yo_proj(om_tile, t_out):
            for n in range(2):
                ws = wslab.tile([128, KT, 128], f32r, tag="wslab", name="wos")
                nc.sync.dma_start(ws[:], WO[n])
                p = psA.tile([128, TOK], f32, tag="mm", name="pyo")
                for k in range(KT):
                    mm(p[:], ws[:, k, :], om_tile[:, k, :], k == 0, k == KT - 1)
                o = stg.tile([128, TOK], f16, tag="io", name="yo", bufs=1)
                nc.vector.tensor_copy(o[:], p[:])
                nc.scalar.dma_start(YO[t_out, n * 128:(n + 1) * 128, :], o[:])

        xk0, qT = x_side_qk(0)
        kxT, vx = x_side_kv(xk0)

        for t in range(NCH):
            # ---- wm projection + AG1 (f16 payload)
            omps = proj_nslab(WM, om)
            agin1 = dram.tile([FPC, TOK], f16, name=f"agin1_{t}")
            for n in range(2):
                s = stg.tile([128, TOK], f16, tag="io", name="oml", bufs=1)
                nc.vector.tensor_copy(s[:], omps[n][:])
                nc.scalar.dma_start(agin1[n * 128:(n + 1) * 128, :], s[:])
            agout1 = dram.tile([DIM, TOK], f16, addr_space=SH,
                               name=f"agout1_{t}")
            allgather(agin1, agout1)
            if t > 0:
                yo_proj(om, t - 1)
            if t + 1 < NCH:
                xk_n, qT_n = x_side_qk(t + 1)
            om1 = big.tile([128, KT, TOK], f32r, tag="big", name="om1")
            nc.gpsimd.dma_start(om1[:],
                                agout1[:].rearrange("(k p) t2 -> p k t2", p=128))
            # ---- norm1 scale (commutes through W1/W3) + FFN up
            bc1 = smol.tile([128, TOK], f16, tag="bc1", name="bc1")
            rowscale(om1, bc1)
            g = gpool.tile([128, HKT, TOK], f32r, tag="g", name="g")
            for n in range(HKT):
                w1s = wslab.tile([128, KT, 128], f32r, tag="wslab", name="w1s")
                nc.sync.dma_start(w1s[:], W1[n])
                w3s = wslab.tile([128, KT, 128], f32r, tag="wslab", name="w3s")
                nc.sync.dma_start(w3s[:], W3[n])
                p1 = psA.tile([128, TOK], f32, tag="mm", name="p1")
                p3 = psA.tile([128, TOK], f32, tag="mm", name="p3")
                for k in range(KT):
                    mm(p1[:], w1s[:, k, :], om1[:, k, :], k == 0, k == KT - 1)
                for k in range(KT):
                    mm(p3[:], w3s[:, k, :], om1[:, k, :], k == 0, k == KT - 1)
                e1 = stg.tile([128, TOK], f32, tag="scr", name="e1")
                nc.vector.tensor_mul(e1[:], p1[:], bc1[:])
                sil = stg.tile([128, TOK], f32, tag="scr", name="sil")
                nc.scalar.activation(sil[:], e1[:], AFT.Silu)
                e3 = stg.tile([128, TOK], f32, tag="scr", name="e3")
                nc.vector.tensor_mul(e3[:], p3[:], bc1[:])
                nc.vector.tensor_mul(g[:, n, :], sil[:], e3[:])
            # ---- FFN down + fused residual + split AR (f16 payload)
            arouts = []
            for half in range(2):
                arin = dram.tile([DIM // 2, TOK], f16,
                                 name=f"arin_{t}_{half}")
                for i in range(8):
                    nf = half * 8 + i
                    w2s = w2slab.tile([128, HKT, 128], f32r, tag="w2s",
                                      name="w2s")
                    nc.sync.dma_start(w2s[:], W2[nf])
                    p = psA.tile([128, TOK], f32, tag="mm", name="pd")
                    for k in range(HKT):
                        mm(p[:], w2s[:, k, :], g[:, k, :], k == 0, k == HKT - 1)
                    o = stg.tile([128, TOK], f16, tag="scr", name="fo")
                    nc.vector.scalar_tensor_tensor(
                        o[:], om1[:, nf, :], 1.0 / NC, p[:],
                        op0=AluOp.mult, op1=AluOp.add)
                    nc.scalar.dma_start(arin[i * 128:(i + 1) * 128, :], o[:])
                arout = dram.tile([DIM // 2, TOK], f16, addr_space=SH,
                                  name=f"arout_{t}_{half}")
                allreduce(arin, arout)
                arouts.append(arout)
            if t + 1 < NCH:
                kxT_n, vx_n = x_side_kv(xk_n)
            om2 = big.tile([128, KT, TOK], f32r, tag="big", name="om2")
            for half in range(2):
                nc.gpsimd.dma_start(
                    om2[:, half * 8:(half + 1) * 8, :],
                    arouts[half][:].rearrange("(k p) t2 -> p k t2", p=128))
            # ---- norm2 scale (applied to km scores / att columns) + mem K/V
            bc2 = smol.tile([128, TOK], f16, tag="bc2", name="bc2")
            rowscale(om2, bc2)
            kmps = proj_nslab(WKM, om2)
            kmT = kmpool.tile([128, 2, TOK], f32r, tag="kmT", name="kmT")
            rope2(kmT, kmps, csm)
            vm = vproj(om2, WVM, "vm")
            # ---- attention (chunk queries only)
            aout = stg.tile([128, 2, TOK], f16, tag="aout", name="aout",
                            bufs=1)
            for h_ in range(HPC):
                pkm = psS.tile([128, TOK], f32, tag="sc", name="pkm")
                pkx = psS.tile([128, TOK], f32, tag="sc", name="pkx")
                for b in range(B):
                    bs = slice(b * 128, (b + 1) * 128)
                    mm(pkm[:, bs], qT[:, h_, bs], kmT[:, h_, bs], True, True)
                    mm(pkx[:, bs], qT[:, h_, bs], kxT[:, h_, bs], True, True)
                s = attp.tile([128, 2, TOK], f32, tag="s", name="s", bufs=1)
                nc.vector.tensor_mul(s[:, 0, :], pkm[:], bc2[:])
                nc.vector.tensor_add(s[:, 1, :], pkx[:], maskx[:])
                # scores are O(1) (max ~5 across the run): exp cannot
                # overflow, masked -6e4 underflows to 0 -> skip max-subtract
                e = attp.tile([128, 2, TOK], f32, tag="e", name="e", bufs=1)
                nc.scalar.activation(e[:], s[:], AFT.Exp)
                # softmax denominator must stay per batch-block
                dn24 = smol.tile([128, 2, B], f32, tag="dn24", name="dn24",
                                 bufs=2)
                nc.vector.tensor_reduce(
                    dn24[:], e[:].rearrange("p h (b t) -> p h b t", b=B),
                    mybir.AxisListType.X, AluOp.add)
                den4 = smol.tile([128, B], f32, tag="den4", name="den4",
                                 bufs=2)
                nc.vector.tensor_reduce(den4[:],
                                        dn24[:].rearrange("p h b -> p b h"),
                                        mybir.AxisListType.X, AluOp.add)
                rec4 = smol.tile([128, B], f32, tag="rec4", name="rec4",
                                 bufs=2)
                nc.vector.reciprocal(rec4[:], den4[:])
                att = attp.tile([128, 2, TOK], f32r, tag="s", name="att",
                                bufs=1)
                for b in range(B):
                    bs = slice(b * 128, (b + 1) * 128)
                    nc.vector.scalar_tensor_tensor(
                        att[:, 0, bs], e[:, 0, bs], rec4[:, b:b + 1],
                        bc2[:, bs], op0=AluOp.mult, op1=AluOp.mult)
                    nc.vector.tensor_scalar_mul(att[:, 1, bs], e[:, 1, bs],
                                                rec4[:, b:b + 1])
                for b in range(B):
                    bs = slice(b * 128, (b + 1) * 128)
                    attT = attp.tile([128, 2, 128], f32r, tag="attT",
                                     name="attT", bufs=2)
                    for half in range(2):
                        pt = psS.tile([128, 128], f32r, tag="tr", name="pt")
                        nc.tensor.transpose(pt[:], att[:, half, bs], ident[:])
                        nc.vector.tensor_copy(attT[:, half, :], pt[:])
                    po = psS.tile([128, 128], f32, tag="tr", name="po")
                    mm(po[:], vm[:, b, h_ * 128:(h_ + 1) * 128],
                       attT[:, 0, :], True, False)
                    mm(po[:], vx[:, b, h_ * 128:(h_ + 1) * 128],
                       attT[:, 1, :], False, True)
                    nc.vector.tensor_copy(aout[:, h_, bs], po[:])
            # ---- AG3 (f16) -> next om
            agin3 = dram.tile([FPC, TOK], f16, name=f"agin3_{t}")
            for h_ in range(HPC):
                nc.scalar.dma_start(agin3[h_ * 128:(h_ + 1) * 128, :],
                                    aout[:, h_, :])
            agout3 = dram.tile([DIM, TOK], f16, addr_space=SH,
                               name=f"agout3_{t}")
            allgather(agin3, agout3)
            om = big.tile([128, KT, TOK], f32r, tag="big", name=f"om_{t + 1}")
            nc.gpsimd.dma_start(om[:],
                                agout3[:].rearrange("(k p) t2 -> p k t2", p=128))
            if t + 1 < NCH:
                qT, kxT, vx = qT_n, kxT_n, vx_n
        yo_proj(om, NCH - 1)
        es.close()

    nc.compile()
    return nc


def _get_runtime():
    if "nc" not in _RUNTIME:
        _RUNTIME["nc"] = _build()
    return _RUNTIME["nc"]


def _assemble(results):
    out = np.zeros((B, SEQ, DIM), np.float32)
    for c in range(NC):
        yo = np.asarray(results[c]["YO"], np.float32)  # [NCH, FPC, TOK]
        y = yo.reshape(NCH, FPC, B, MEM).transpose(2, 0, 3, 1)
        out[:, :, c * FPC:(c + 1) * FPC] = y.reshape(B, SEQ, FPC)
    return out


def kernel(**inputs):
    from concourse.bass_utils import run_bass_kernel_spmd
    nc = _get_runtime()
    in_maps = _prepare(inputs)
    res = run_bass_kernel_spmd(nc, in_maps, core_ids=list(range(NC)),
                               trace=False)
    return _assemble(res.results)


if __name__ == "__main__":
    _build()
    print("build ok")
